# revision 28
# baseline (speedup 1.0000x reference)
"""Trainium2 Bass kernel for nn_ContrastiveLoss_66030827208766.

Strategy (data-parallel over images, captions replicated):
  - 8 cores, 16 images each.  Images are assigned to cores by GLOBAL
    length rank (core = rank % 8, slot = rank // 8), so every core's
    slot-k image has nearly the same valid-object count.  Only valid
    objects are shipped; slots are partitioned into width-groups by a
    small DP that minimizes padded columns plus a per-group instruction
    penalty (the DVE pays ~162 ns per extra reduce per PSUM tile).
  - Padding replicates object 0 (always valid), so a plain max over the
    padded block equals the masked max over valid objects.
  - All matmul operands are bf16 (PE accumulates fp32).  Captions are
    replicated to every core in D-major layout [D, w*128 + c], each
    128-column slice one caption word across all 128 captions, and are
    PRE-SCALED by 1/caption_len on the host (max and sum commute with a
    positive per-caption scalar), which removes the on-device scale.
  - Device per core: per caption word w, one matmul (stationary caption
    chunk [D,128], moving packed image-objects [D, C]) -> one PSUM
    bank; psA (4 banks) / psB (3 banks) ping-pong, 1 junk bank.
    Grouped strided reduce_max per width-group (DVE, the only engine
    with free-axis pooling on TRN2) -> buf[c, slot, w]; final
    contiguous reduce_sum over w -> sout [128 caps, 32 slots] -> one
    DMA out.
  - Host: unpermute slots of each part, add, then the (tiny) triplet
    margin loss reduction in numpy.

Codegen constraint: every TPB instruction can carry at most ONE sync
wait.  Three tactics keep us within it: (1) freshly-DMA'd tiles are
first touched by degenerate 1x1 "junk" matmuls so the real matmuls'
DMA-queue requirements are already observed by the PE; (2) buffers are
laid out so each writer hits a disjoint range (no spurious WAW chains);
(3) a post-pass strips waits that are redundant by construction
(same-engine in-order completion, per-queue DMA FIFO, barrier-covered
drain waits).
"""

import ml_dtypes
import numpy as np

import concourse.bass as bass
import concourse.mybir as mybir
from concourse import tile
from concourse.bass_utils import run_bass_kernel_spmd
from concourse.tile_rust import add_dep_helper

B = 128          # batch (images == captions)
O1, W1 = 36, 50  # part 1: im objects, s words
O2, W2 = 25, 30  # part 2: pred objects, c_r words
D = 128
NCORES = 8
IPC = B // NCORES  # images (slots) per core
MARGIN = 0.2
F32 = mybir.dt.float32
BF16 = mybir.dt.bfloat16

LAST_RESULT = None   # BassKernelResults of the most recent run (for test.py)
_NC = None           # cached program
_NC_KEY = None       # plan key the cached program was built for


def _build_part(nc, pending, hoist, cap, imt, buf, ps_tiles,
                W, groups, C, piece_of, cap_key, dve_chain):
    """Emit matmul + grouped-reduce stream for one t2i part.

    ps_tiles = [(psA, 4), (psB, 3)], alternated; each chunk (= one
    caption word) occupies one PSUM bank.  groups = ((s0, s1, wid), ...)
    partitions the 16 slots into uniform-width runs.  buf is slot-major
    [B, IPC, W].
    """
    w = 0
    t_idx = 0
    while w < W:
        ps, cap_chunks = ps_tiles[t_idx % len(ps_tiles)]
        n = min(cap_chunks, W - w)
        t_idx += 1
        for j in range(n):
            hoist((cap_key, piece_of[w + j]))
            cs = cap[:, (w + j) * B:(w + j + 1) * B]
            mm = nc.tensor.matmul(ps[:, j * 512:j * 512 + C], cs, imt[:],
                                  start=True, stop=True)
            while pending:
                add_dep_helper(mm.ins, pending.pop().ins, sync=False,
                               reason="order matmul after wait-carrier")
        # One grouped reduce per width-group covering all n chunks.
        # c-outer walk + contiguous innermost output (w-major buf) is the
        # fast DVE pattern (~1.03 ns/elem); any strided-innermost output
        # or per-group bank hopping costs ~30%.  The no-sync chain pins
        # the DVE queue to program order -- otherwise the Tile scheduler
        # interleaves the psA/psB tiles' reduces, delaying each PSUM
        # tile's release by ~1.4 us per pair.
        v = ps[:, :n * 512].rearrange("p (c x) -> p c x", c=n)
        off = 0
        for s0, s1, wid in groups:
            r = nc.vector.reduce_max(
                buf[:, w:w + n, s0:s1],
                v[:, :, off:off + (s1 - s0) * wid].rearrange(
                    "p c (g o) -> p c g o", o=wid),
                axis=mybir.AxisListType.X,
            )
            if dve_chain[0] is not None:
                add_dep_helper(r.ins, dve_chain[0].ins, sync=False,
                               reason="pin DVE queue order")
            dve_chain[0] = r
            off += (s1 - s0) * wid
        w += n


def _build_nc(plan_key):
    groups1, groups2 = plan_key
    C1 = sum((s1 - s0) * wid for s0, s1, wid in groups1)
    C2 = sum((s1 - s0) * wid for s0, s1, wid in groups2)
    # Shrink the bass-managed semaphore ID space: the walrus-generated
    # NEFF teardown resets every declared semaphore one instruction at a
    # time (~140 ns each, ~250 sems = ~8 us of fixed tail).  This kernel
    # uses ~15 sems; declaring 54..119 instead of 54..255 more than
    # halves that teardown.
    saved = bass.get_kernel_semaphore_range
    bass.get_kernel_semaphore_range = lambda: range(saved().start, 170)
    try:
        nc = bass.Bass()
    finally:
        bass.get_kernel_semaphore_range = saved
    capT1 = nc.dram_tensor("capT1", [D, B * W1], BF16, kind="ExternalInput")
    capT2 = nc.dram_tensor("capT2", [D, B * W2], BF16, kind="ExternalInput")
    imT1 = nc.dram_tensor("imT1", [D, C1], BF16, kind="ExternalInput")
    imT2 = nc.dram_tensor("imT2", [D, C2], BF16, kind="ExternalInput")
    out_t = nc.dram_tensor("scores_t", [B, 2 * IPC], F32,
                           kind="ExternalOutput")

    # cap pieces: part 1 in 6 pieces (small ones first so the first PSUM
    # tile's matmuls start as early as possible), part 2 in 3 pieces of
    # 10 words, interleaved so cap2 lands before part 2 starts.
    P1_SPLITS = (0, 4, 10, 20, 30, 40, 50)   # word boundaries of cap1 pieces
    NP2 = 3
    PW2 = W2 // NP2

    with tile.TileContext(nc) as tc:
        with (
            tc.tile_pool(name="const", bufs=1) as cpool,
            tc.tile_pool(name="psum", bufs=1, space="PSUM") as pspool,
            tc.tile_pool(name="work", bufs=1) as wpool,
        ):
            # ---- input DMAs, alternating the two HWDGE rings (even
            # emission -> sync, odd -> scalar).  DMAHW bookkeeping lanes
            # are assigned by global round-robin, so each of the 8 lanes
            # sees a single issuing engine -> per-lane FIFO holds and
            # own-lane waits are strippable.  The output DMA is emission
            # #10 -> lane 2 (sync), same engine as lane 2's input.
            dma_idx = [0]

            def load(dst_ap, src_ap):
                eng = nc.sync if dma_idx[0] % 2 == 0 else nc.scalar
                dma_idx[0] += 1
                return eng.dma_start(dst_ap, src_ap)

            imt1 = cpool.tile([D, C1], BF16, tag="imt1")
            cap1 = cpool.tile([D, B * W1], BF16, tag="cap1")
            imt2 = cpool.tile([D, C2], BF16, tag="imt2")
            cap2 = cpool.tile([D, B * W2], BF16, tag="cap2")

            def p1(j):
                a, b = P1_SPLITS[j] * B, P1_SPLITS[j + 1] * B
                load(cap1[:, a:b], capT1[:, a:b])

            def p2(j):
                c = PW2 * B
                load(cap2[:, j * c:(j + 1) * c], capT2[:, j * c:(j + 1) * c])

            # emission order: e0 imt1(sy) e1 c1p0(sc) e2 c1p1(sy)
            # e3 c1p2(sc) e4 c1p3(sy) e5 c2p0(sc) e6 c1p4(sy)
            # e7 c2p1(sc) e8 c1p5(sy) e9 c2p2(sc) e10 imt2(sy); the
            # output DMA is emission #11 -> lane 3 (scalar), same engine
            # as lane 3's input.
            load(imt1[:], imT1[:])
            p1(0)
            p1(1)
            p1(2)
            p1(3)
            p2(0)
            p1(4)
            p2(1)
            p1(5)
            p2(2)
            load(imt2[:], imT2[:])
            assert dma_idx[0] == 11, dma_idx

            # w-major: reduce_max outputs are contiguous (innermost =
            # slot within a group run); the final strided reduce_sum
            # costs ~0.7us more but the max-reduces save ~8us.
            buf1 = wpool.tile([B, W1, IPC], F32, tag="buf1")
            buf2 = wpool.tile([B, W2, IPC], F32, tag="buf2")

            # Static PSUM: psA (4 banks) + psB (3 banks) ping-pong,
            # 1 junk bank.
            psA = pspool.tile([B, 2048], F32, tag="psA", name="psA")
            psB = pspool.tile([B, 1536], F32, tag="psB", name="psB")
            junk_ps = pspool.tile([1, 1], F32, tag="junk_ps", name="junk_ps")

            hoisted = {}
            pending = []
            corners = {
                "imt1": imt1[:1, :1],
                "imt2": imt2[:1, :1],
            }
            for j in range(len(P1_SPLITS) - 1):
                corners[("cap1", j)] = cap1[:1, P1_SPLITS[j] * B:
                                            P1_SPLITS[j] * B + 1]
            for j in range(NP2):
                corners[("cap2", j)] = cap2[:1, j * PW2 * B:j * PW2 * B + 1]
            piece_of1 = []
            for j in range(len(P1_SPLITS) - 1):
                piece_of1 += [j] * (P1_SPLITS[j + 1] - P1_SPLITS[j])
            piece_of2 = [w // PW2 for w in range(W2)]

            def hoist(key):
                if key in hoisted:
                    return
                ap = corners[key]
                hoisted[key] = nc.tensor.matmul(
                    junk_ps[:, :], ap, ap,
                    start=True, stop=True, skip_group_check=True,
                )
                pending.append(hoisted[key])

            sout = wpool.tile([B, 2 * IPC], F32, tag="sout")

            ps_tiles = [(psA, 4), (psB, 3)]

            dve_chain = [None]

            def chained_sum(dst, src):
                r = nc.vector.reduce_sum(dst, src, axis=mybir.AxisListType.X)
                add_dep_helper(r.ins, dve_chain[0].ins, sync=False,
                               reason="pin DVE queue order")
                dve_chain[0] = r
                return r

            hoist("imt1")
            _build_part(nc, pending, hoist, cap1, imt1, buf1, ps_tiles,
                        W1, groups1, C1, piece_of1, "cap1", dve_chain)
            # Part-1 epilogue emitted before part 2: the DVE executes its
            # queue in order, so this overlaps part-2 matmuls.
            chained_sum(sout[:, :IPC], buf1[:].rearrange("p w i -> p i w"))

            hoist("imt2")
            _build_part(nc, pending, hoist, cap2, imt2, buf2, ps_tiles,
                        W2, groups2, C2, piece_of2, "cap2", dve_chain)
            chained_sum(sout[:, IPC:], buf2[:].rearrange("p w i -> p i w"))
            out_dma = nc.scalar.dma_start(out_t[:], sout[:])

    # ---- wait-strip post-pass ----------------------------------------
    # Walrus codegen accepts at most one sync wait per instruction;
    # remove waits that are redundant by construction.
    out_q = {u.ant_name for u in out_dma.ins.sync_info.on_update
             if u.ant_name.startswith("DMAHW")}
    for bb in nc.main_func.blocks:
        for ins in bb.instructions:
            si = ins.sync_info
            if si is None:
                continue
            t = type(ins).__name__
            if t == "InstDrain" and len(si.on_wait) > 2:
                # Kernel-tail drain: engine completion is enforced by the
                # per-engine drains + EVSEM butterfly that follow, and
                # input-DMA completions are covered transitively by the
                # compute that consumed them.  Only the output DMA's
                # queue wait is load-bearing.
                drop = lambda w: w.ant_name not in out_q
            elif t == "InstMatmult":
                # WAW on a reused psum bank: the prior matmul's drain
                # (~128 cyc) finished >=2 matmul-streams earlier, so the
                # same-engine completion wait is dead.
                drop = lambda w: w.ant_name.startswith("PE_")
            elif getattr(ins, "engine", None) == mybir.EngineType.DVE:
                # DVE fully drains its pipe between ops; waits on earlier
                # DVE completions are satisfied at issue.
                drop = lambda w: w.ant_name.startswith("DVE_")
            elif t == "InstDMACopy":
                # Per-lane FIFO (single issuing engine per lane by
                # construction) makes own-lane waits redundant.
                own = {u.ant_name for u in si.on_update
                       if u.ant_name.startswith("DMAHW")}
                drop = lambda w: w.ant_name in own
            else:
                continue
            kept = [w for w in si.on_wait if not drop(w)]
            if len(kept) != len(si.on_wait):
                si.on_wait = kept
                ins.sync_info = si
    return nc


# DVE cost of one extra reduce instruction per PSUM tile, expressed in
# packed columns (162 ns overhead / 1.042 ns-per-col / 3.5 words-per-tile).
_LAMBDA_COLS = 44


def _plan(lens, omax):
    """Global length-rank plan: order[r] = image of rank r; core r%8 slot
    r//8.  DP-partition the 16 slots into uniform-width groups
    minimizing padded columns + _LAMBDA_COLS per extra group."""
    lens = np.clip(np.asarray(lens, dtype=np.int64), 1, omax)
    order = np.argsort(lens, kind="stable")
    L = lens[order]
    # width of a group ending at slot b (exclusive) = len at global rank
    # 8b-1 (the longest image among its slots, identical across cores).
    gw = [int(L[NCORES * b - 1]) for b in range(IPC + 1)]  # gw[b], b=1..16

    INF = 1 << 40
    cost = [INF] * (IPC + 1)
    prev = [0] * (IPC + 1)
    cost[0] = -_LAMBDA_COLS  # first group carries no penalty
    for b in range(1, IPC + 1):
        for a in range(b):
            c = cost[a] + (b - a) * gw[b] + _LAMBDA_COLS
            if c < cost[b]:
                cost[b] = c
                prev[b] = a
    groups = []
    b = IPC
    while b > 0:
        a = prev[b]
        groups.append((a, b, gw[b]))
        b = a
    groups.reverse()
    C = sum((s1 - s0) * wid for s0, s1, wid in groups)
    assert C <= 512, (C, groups)
    return order, tuple(groups)


def _pack_images(x_bf, lens, order, groups, core):
    """Build the packed, padded, D-major [D, C] bf16 image-object matrix
    for one core.  Slot k = image order[8k + core]; its first lens[i]
    objects, padded to its group width by replicating object 0."""
    widths = {}
    for s0, s1, wid in groups:
        for k in range(s0, s1):
            widths[k] = wid
    cols = []
    for k in range(IPC):
        i = order[NCORES * k + core]
        wid = widths[k]
        L = min(int(lens[i]), wid)
        blk = np.empty((wid, D), dtype=x_bf.dtype)
        blk[:L] = x_bf[i, :L]
        blk[L:] = x_bf[i, 0]
        cols.append(blk)
    return np.ascontiguousarray(np.concatenate(cols, axis=0).T)


def kernel(im, im_l, s, s_l, pred, pred_l, cap_o_pred, cap_o_l, c_r_pred,
           c_r_l, trace=False, tmpdir=None):
    global LAST_RESULT, _NC, _NC_KEY
    im = np.asarray(im, dtype=np.float32)
    s = np.asarray(s, dtype=np.float32)
    pred = np.asarray(pred, dtype=np.float32)
    c_r_pred = np.asarray(c_r_pred, dtype=np.float32)
    im_l = np.asarray(im_l)
    pred_l = np.asarray(pred_l)

    order1, groups1 = _plan(im_l, O1)
    order2, groups2 = _plan(pred_l, O2)
    plan_key = (groups1, groups2)

    im_bf = im.astype(ml_dtypes.bfloat16)
    pred_bf = pred.astype(ml_dtypes.bfloat16)

    def dmajor16(x, inv_len):
        # pre-scale each caption's words by 1/len, then D-major bf16
        x = x * inv_len[:, None, None]
        b, w, d = x.shape
        t = np.ascontiguousarray(x.transpose(1, 0, 2).reshape(w * b, d).T)
        return t.astype(ml_dtypes.bfloat16)

    capT1 = dmajor16(s, 1.0 / np.asarray(s_l, dtype=np.float32))
    capT2 = dmajor16(c_r_pred, 1.0 / np.asarray(c_r_l, dtype=np.float32))

    in_maps = []
    for m in range(NCORES):
        in_maps.append({
            "capT1": capT1,
            "capT2": capT2,
            "imT1": _pack_images(im_bf, im_l, order1, groups1, m),
            "imT2": _pack_images(pred_bf, pred_l, order2, groups2, m),
        })

    if _NC is None or _NC_KEY != plan_key:
        _NC = _build_nc(plan_key)
        _NC_KEY = plan_key
    res = run_bass_kernel_spmd(_NC, in_maps, list(range(NCORES)), trace=trace,
                               tmpdir=tmpdir)
    LAST_RESULT = res

    # Each core returns [128 caps, 32]: part-1 slots then part-2 slots,
    # already scaled by 1/caption_len.  Unpermute slots back to image
    # order and add the parts.
    scores = np.zeros((B, B), dtype=np.float32)
    for m in range(NCORES):
        tile_m = res.results[m]["scores_t"]  # [128, 32]
        idx1 = order1[np.arange(IPC) * NCORES + m]
        idx2 = order2[np.arange(IPC) * NCORES + m]
        scores[idx1, :] += tile_m[:, :IPC].T
        scores[idx2, :] += tile_m[:, IPC:].T

    # Triplet margin loss on the full (tiny) B x B matrix.
    d = np.diag(scores).copy()
    cost_s = np.maximum(MARGIN + scores - d[:, None], 0.0).astype(np.float32)
    cost_im = np.maximum(MARGIN + scores - d[None, :], 0.0).astype(np.float32)
    np.fill_diagonal(cost_s, 0.0)
    np.fill_diagonal(cost_im, 0.0)
    out = cost_s.max(axis=1).sum() + cost_im.max(axis=0).sum()
    return np.asarray(out, dtype=np.float32)
